# revision 20
# baseline (speedup 1.0000x reference)
"""Cross-attention kernel for 8 Trainium2 NeuronCores.

Problem: nn_CrossAttention (N=2, X=1024, T=4096, D=1024, H=16, hd=64).

Sharding: core c handles batch n = c//4 and head-group hg = c%4
(4 heads = 256 output dims). No cross-core communication.

Host prep per core (numpy, outside HW timing):
  - xT   = previous_output[n].T          (D, X)  bf16
  - ctxT = context[n].T                  (D, T)  bf16
  - w{q,k,v}T = W[256*hg:256*(hg+1)].T   (D, 256) bf16
  - biases sliced per core (bv replicated to 128 partitions).

Device (all matmuls contract over the partition dim):
  qT[c,x]  = wqT.T @ xT    (+bq)         kT[c,t] = wkT.T @ ctxT (+bk)
  v[t,c]   = ctxT.T @ wvT  (+bv via DVE broadcast add)
  S.T[t,x] = kT_h.T @ qT_h   (per head, K=64, head pairs packed into
                              array row-halves via base_partition)
  P.T      = exp(S.T / 8)                 (ScalarE, scale folded in)
  O'.T[65,x] = [V_h | 1].T @ P.T          (ones col gives softmax denom)
  O[x,64]  = transpose(O'.T) rows 0:64 * 1/row64   (PE transpose + DVE)

The program is emitted as one software pipeline so exp (ScalarE,
~147us/core total) overlaps the PE work (~165us/core total):
  era1: per ctx chunk c: kT ct0 chunk + per tt: v[tt] + attention
        steps (hp=0, xc=0) and (hp=0, xc=1)
  era2: per tt: kT ct1 pacing + attention steps (hp=1, xc=0/1)
"""

import numpy as np
import ml_dtypes
from contextlib import ExitStack

import concourse.bass as bass
import concourse.bacc as bacc
import concourse.tile as tile
import concourse.mybir as mybir
from concourse.bass_utils import run_bass_kernel_spmd
from concourse.masks import make_identity

D, H, HD = 1024, 16, 64
N, X, T = 2, 1024, 4096
NCORES = 8
CH = 4            # heads per core
CW = CH * HD      # 256 output cols per core
KT = D // 128     # 8 d-tiles
TT = T // 128     # 32 t-tiles
XTILES = X // 128  # 8 x-tiles
BF16 = mybir.dt.bfloat16
F32 = mybir.dt.float32
EXP = mybir.ActivationFunctionType.Exp

_CACHE = {}


def _build_program():
    nc = bacc.Bacc("TRN2", target_bir_lowering=False, debug=False,
                   num_devices=NCORES)

    # layouts are pre-swizzled on the host so every DMA row is contiguous
    xt_d = nc.dram_tensor("xt", (2, 128, KT, 512), BF16, kind="ExternalInput")
    ctxt_d = nc.dram_tensor("ctxt", (8, 128, KT, 512), BF16,
                            kind="ExternalInput")
    wqt_d = nc.dram_tensor("wqt", (128, KT, CW), BF16, kind="ExternalInput")
    wkt_d = nc.dram_tensor("wkt", (128, KT, CW), BF16, kind="ExternalInput")
    wvt_d = nc.dram_tensor("wvt", (128, KT, CW), BF16, kind="ExternalInput")
    bq_d = nc.dram_tensor("bq", (128, 2), F32, kind="ExternalInput")
    bk_d = nc.dram_tensor("bk", (128, 2), F32, kind="ExternalInput")
    bv_d = nc.dram_tensor("bv", (128, CW), BF16, kind="ExternalInput")
    out_d = nc.dram_tensor("out", (X, CW), F32, kind="ExternalOutput")

    with tile.TileContext(nc) as tc, ExitStack() as ctx:
        consts = ctx.enter_context(tc.tile_pool(name="consts", bufs=1))
        pt_pool = ctx.enter_context(tc.tile_pool(name="pt", bufs=3))
        osb_pool = ctx.enter_context(tc.tile_pool(name="osb", bufs=2))
        rc_pool = ctx.enter_context(tc.tile_pool(name="rc", bufs=2))
        # one psum pool for everything except the score tiles:
        # 4 slots x 1 bank (projections, O' accumulators, transposes)
        mp = ctx.enter_context(tc.tile_pool(name="mp", bufs=4, space="PSUM"))
        # score tiles: 2 slots x 2 banks (double-buffered so ScalarE's exp
        # never gates the next score matmul pair)
        st_pool = ctx.enter_context(
            tc.tile_pool(name="st", bufs=2, space="PSUM"))

        # ---- resident SBUF tensors ----
        wq_sb = consts.tile([128, KT, CW], BF16)
        wk_sb = consts.tile([128, KT, CW], BF16)
        wv_sb = consts.tile([128, KT, CW], BF16)
        xt_sb = consts.tile([128, KT, X], BF16)
        ctx_sb = consts.tile([128, KT, T], BF16)
        qt_sb = consts.tile([128, 2, X], BF16)
        kt_sb = consts.tile([128, 2, T], BF16)
        vp_sb = consts.tile([128, TT, CH * (HD + 1)], BF16)  # [.., 260]
        out_sb = consts.tile([128, XTILES, CW], F32)
        bq_sb = consts.tile([128, 2], F32)
        bk_sb = consts.tile([128, 2], F32)
        bv_sb = consts.tile([128, CW], BF16)
        ident = consts.tile([128, 128], F32)

        vp_h = vp_sb[:].rearrange("p t (h c) -> p t h c", c=HD + 1)
        bv_h = bv_sb[:].rearrange("p (h c) -> p h c", c=HD)

        # ---- PE warm-up: dummy matmuls while input DMAs land (HAM) ----
        dumin = consts.tile([128, 512], BF16)
        nc.gpsimd.memset(dumin[:], 0.0)
        dps = mp.tile([128, 512], F32, tag="mp", name="dps")
        for i in range(10):
            nc.tensor.matmul(dps[:], dumin[:, 0:128], dumin[:],
                             start=(i == 0), stop=(i == 9))

        # ---- input DMAs: two queues so the k/v context stream and the
        # q-side stream transfer in parallel ----
        def ctx_dma(c):
            nc.gpsimd.dma_start(ctx_sb[:, :, 512 * c:512 * (c + 1)],
                                ctxt_d.ap()[c])

        nc.gpsimd.dma_start(wk_sb[:], wkt_d.ap())
        ctx_dma(0)
        nc.gpsimd.dma_start(wv_sb[:], wvt_d.ap())
        nc.sync.dma_start(xt_sb[:, :, 0:512], xt_d.ap()[0])
        nc.sync.dma_start(wq_sb[:], wqt_d.ap())
        nc.sync.dma_start(bq_sb[:], bq_d.ap())
        nc.sync.dma_start(bk_sb[:], bk_d.ap())
        nc.sync.dma_start(bv_sb[:], bv_d.ap())
        ctx_dma(1)
        nc.sync.dma_start(xt_sb[:, :, 512:1024], xt_d.ap()[1])
        for c in range(2, 8):
            ctx_dma(c)
        make_identity(nc, ident[:])
        nc.gpsimd.memset(vp_h[:, :, :, HD:HD + 1], 1.0)

        # ---- qT projection: [col, x] per (col-tile, x-chunk) ----
        def qt_proj(ct, xc):
            ps = mp.tile([128, 512], F32, tag="mp", name=f"qps{ct}{xc}")
            for dt in range(KT):
                nc.tensor.matmul(
                    ps[:],
                    wq_sb[:, dt, 128 * ct:128 * (ct + 1)],
                    xt_sb[:, dt, 512 * xc:512 * (xc + 1)],
                    start=(dt == 0), stop=(dt == KT - 1))
            nc.vector.tensor_scalar_add(
                qt_sb[:, ct, 512 * xc:512 * (xc + 1)], ps[:],
                bq_sb[:, ct:ct + 1])

        qt_proj(0, 0)   # the other three slices ride later as stream filler

        def kt_chunk(ct, c):
            ps = mp.tile([128, 512], F32, tag="mp", name=f"kps{ct}_{c}")
            for dt in range(KT):
                nc.tensor.matmul(
                    ps[:],
                    wk_sb[:, dt, 128 * ct:128 * (ct + 1)],
                    ctx_sb[:, dt, 512 * c:512 * (c + 1)],
                    start=(dt == 0), stop=(dt == KT - 1))
            nc.vector.tensor_scalar_add(
                kt_sb[:, ct, 512 * c:512 * (c + 1)], ps[:],
                bk_sb[:, ct:ct + 1])

        def v_tile(tt):
            ps = mp.tile([128, 512], F32, tag="mp", name=f"vps{tt}")
            for dt in range(KT):
                nc.tensor.matmul(
                    ps[:, 0:CW],
                    ctx_sb[:, dt, 128 * tt:128 * (tt + 1)],
                    wv_sb[:, dt, :],
                    start=(dt == 0), stop=(dt == KT - 1))
            nc.vector.tensor_add(
                vp_h[:, tt, :, 0:HD],
                ps[:, 0:CW].rearrange("p (h c) -> p h c", c=HD),
                bv_h[:])

        # attention state
        oacc = {}     # (hp, xc) -> [tileA, tileB]

        def attn_start(hp, xc):
            oacc[(hp, xc)] = [
                mp.tile([65, 512], F32, tag="mp", name=f"oacc{hp}{xc}{h2}")
                for h2 in range(2)]

        def attn_step(hp, xc, tt):
            st = st_pool.tile([128, 1024], F32, tag="st", name=f"st{hp}{xc}{tt}")
            for h2 in range(2):
                nc.tensor.matmul(
                    st[:, 512 * h2:512 * (h2 + 1)],
                    kt_sb[64 * h2:64 * (h2 + 1), hp,
                          128 * tt:128 * (tt + 1)],
                    qt_sb[64 * h2:64 * (h2 + 1), hp,
                          512 * xc:512 * (xc + 1)],
                    start=True, stop=True)
            pt = pt_pool.tile([128, 1024], BF16, tag="pt", name=f"pt{hp}{xc}{tt}")
            nc.scalar.activation(pt[:], st[:], EXP, scale=0.125)
            for h2 in range(2):
                h = 2 * hp + h2
                nc.tensor.matmul(
                    oacc[(hp, xc)][h2][:],
                    vp_sb[:, tt, 65 * h:65 * (h + 1)],
                    pt[:, 512 * h2:512 * (h2 + 1)],
                    start=(tt == 0), stop=(tt == TT - 1))

        def attn_drain(hp, xc, out_ap=None):
            ots = []
            for h2 in range(2):
                ot = osb_pool.tile([65, 512], F32, tag="osb", name=f"ot{hp}{xc}{h2}")
                nc.vector.tensor_copy(ot[:], oacc[(hp, xc)][h2][:])
                ots.append(ot)
            for s in range(4):
                for h2 in range(2):
                    h = 2 * hp + h2
                    tp = mp.tile([128, 65], F32, tag="mp", name=f"tp{hp}{xc}{h2}{s}")
                    nc.tensor.transpose(
                        tp[:], ots[h2][:, 128 * s:128 * (s + 1)],
                        ident[0:65, 0:65])
                    rc = rc_pool.tile([128, 1], F32, tag="rc", name=f"rc{hp}{xc}{h2}{s}")
                    nc.vector.reciprocal(rc[:], tp[:, 64:65])
                    nc.vector.tensor_scalar_mul(
                        out_sb[:, 4 * xc + s, 64 * h:64 * (h + 1)],
                        tp[:, 0:64], rc[:])
                if out_ap is not None:
                    # this stream completes x-tile 4*xc+s: ship it out now
                    nc.sync.dma_start(out_ap[:, 4 * xc + s:4 * xc + s + 1],
                                      out_sb[:, 4 * xc + s:4 * xc + s + 1])
            del oacc[(hp, xc)]

        # One attention stream (hp, xc) at a time; PE filler work
        # (kT chunks, v tiles, qT ct1) rides inside the streams so
        # ScalarE's exp stays busy end to end. Each stream's drain is
        # deferred into the next stream's first steps to hide the
        # inter-stream bubble (the freed O' accumulators supply the
        # PSUM slots the drain's transposes need).
        out_ap = out_d.ap().rearrange("(xt p) c -> p xt c", p=128)

        # stream (0,0): kT ct0 chunk-paced + v paced + qT(0,1) +
        # kT ct1 chunk 0
        attn_start(0, 0)
        for c in range(8):
            kt_chunk(0, c)
            for tt in range(4 * c, 4 * c + 4):
                v_tile(tt)
                attn_step(0, 0, tt)
                if tt == 18:
                    qt_proj(0, 1)
            if c == 7:
                kt_chunk(1, 0)

        # stream (0,1): drain of (0,0) overlapped, kT ct1 chunks 1-3,
        # qT(1,0)
        attn_start(0, 1)
        for tt in range(TT):
            attn_step(0, 1, tt)
            if tt == 2:
                attn_drain(0, 0)
            elif tt in (4, 12, 20):
                kt_chunk(1, 1 + (tt - 4) // 8)
            elif tt == 26:
                qt_proj(1, 0)

        # stream (1,0): kT ct1 chunks 4-7 paced (needed from step 16 on),
        # qT(1,1)
        attn_start(1, 0)
        for tt in range(TT):
            attn_step(1, 0, tt)
            if tt == 2:
                attn_drain(0, 1)
            elif tt in (4, 8, 12, 15):
                kt_chunk(1, 4 + [4, 8, 12, 15].index(tt))
            elif tt == 20:
                qt_proj(1, 1)

        # stream (1,1)
        attn_start(1, 1)
        for tt in range(TT):
            attn_step(1, 1, tt)
            if tt == 2:
                attn_drain(1, 0, out_ap)
        attn_drain(1, 1, out_ap)

    nc.compile()
    return nc


def get_program():
    if "nc" not in _CACHE:
        _CACHE["nc"] = _build_program()
    return _CACHE["nc"]


def _swizzle(at, inner):
    """(D, M) d-major -> (M//inner, 128, KT, inner): chunked, partition-
    contiguous rows so each DMA descriptor is a long linear run."""
    dd, m = at.shape
    return np.ascontiguousarray(
        at.reshape(KT, 128, m // inner, inner).transpose(2, 1, 0, 3))


def _shard_inputs(previous_output, context, Wq, bq, Wk, bk, Wv, bv):
    bf = ml_dtypes.bfloat16
    xt = [_swizzle(previous_output[n].T.astype(bf), 512) for n in range(N)]
    ctxt = [_swizzle(context[n].T.astype(bf), 512) for n in range(N)]
    in_maps = []
    for c in range(NCORES):
        n, hg = c // CH, c % CH
        sl = slice(CW * hg, CW * (hg + 1))
        in_maps.append({
            "xt": xt[n],
            "ctxt": ctxt[n],
            "wqt": _swizzle(Wq[sl].T.astype(bf), CW)[0],
            "wkt": _swizzle(Wk[sl].T.astype(bf), CW)[0],
            "wvt": _swizzle(Wv[sl].T.astype(bf), CW)[0],
            "bq": np.ascontiguousarray(
                bq[sl].reshape(2, 128).T).astype(np.float32),
            "bk": np.ascontiguousarray(
                bk[sl].reshape(2, 128).T).astype(np.float32),
            "bv": np.broadcast_to(
                bv[sl].astype(bf), (128, CW)).copy(),
        })
    return in_maps


LAST_RESULTS = None


def kernel(previous_output, context, Wq, bq, Wk, bk, Wv, bv):
    global LAST_RESULTS
    previous_output = np.asarray(previous_output, dtype=np.float32)
    context = np.asarray(context, dtype=np.float32)
    Wq = np.asarray(Wq, dtype=np.float32)
    Wk = np.asarray(Wk, dtype=np.float32)
    Wv = np.asarray(Wv, dtype=np.float32)
    bq = np.asarray(bq, dtype=np.float32)
    bk = np.asarray(bk, dtype=np.float32)
    bv = np.asarray(bv, dtype=np.float32)

    nc = get_program()
    in_maps = _shard_inputs(previous_output, context, Wq, bq, Wk, bk, Wv, bv)
    res = run_bass_kernel_spmd(nc, in_maps, core_ids=list(range(NCORES)))
    LAST_RESULTS = res

    out = np.empty((N, X, D), dtype=np.float32)
    for c in range(NCORES):
        n, hg = c // CH, c % CH
        out[n, :, CW * hg:CW * (hg + 1)] = res.results[c]["out"]
    return out


# revision 21
# speedup vs baseline: 1.0292x; 1.0292x over previous
"""Cross-attention kernel for 8 Trainium2 NeuronCores.

Problem: nn_CrossAttention (N=2, X=1024, T=4096, D=1024, H=16, hd=64).

Sharding: core c handles batch n = c//4 and head-group hg = c%4
(4 heads = 256 output dims). No cross-core communication.

Host prep per core (numpy, outside HW timing):
  - xT   = previous_output[n].T          (D, X)  bf16
  - ctxT = context[n].T                  (D, T)  bf16
  - w{q,k,v}T = W[256*hg:256*(hg+1)].T   (D, 256) bf16
  - biases sliced per core (bv replicated to 128 partitions).

Device (all matmuls contract over the partition dim):
  qT[c,x]  = wqT.T @ xT    (+bq)         kT[c,t] = wkT.T @ ctxT (+bk)
  v[t,c]   = ctxT.T @ wvT  (+bv via DVE broadcast add)
  S.T[t,x] = kT_h.T @ qT_h   (per head, K=64, head pairs packed into
                              array row-halves via base_partition)
  P.T      = exp(S.T / 8)                 (ScalarE, scale folded in)
  O'.T[65,x] = [V_h | 1].T @ P.T          (ones col gives softmax denom)
  O[x,64]  = transpose(O'.T) rows 0:64 * 1/row64   (PE transpose + DVE)

The program is emitted as one software pipeline so exp (ScalarE,
~147us/core total) overlaps the PE work (~165us/core total):
  era1: per ctx chunk c: kT ct0 chunk + per tt: v[tt] + attention
        steps (hp=0, xc=0) and (hp=0, xc=1)
  era2: per tt: kT ct1 pacing + attention steps (hp=1, xc=0/1)
"""

import numpy as np
import ml_dtypes
from contextlib import ExitStack

import concourse.bass as bass
import concourse.bacc as bacc
import concourse.tile as tile
import concourse.mybir as mybir
from concourse.bass_utils import run_bass_kernel_spmd
from concourse.masks import make_identity

D, H, HD = 1024, 16, 64
N, X, T = 2, 1024, 4096
NCORES = 8
CH = 4            # heads per core
CW = CH * HD      # 256 output cols per core
KT = D // 128     # 8 d-tiles
TT = T // 128     # 32 t-tiles
XTILES = X // 128  # 8 x-tiles
BF16 = mybir.dt.bfloat16
F32 = mybir.dt.float32
EXP = mybir.ActivationFunctionType.Exp

_CACHE = {}


def _build_program():
    nc = bacc.Bacc("TRN2", target_bir_lowering=False, debug=False,
                   num_devices=NCORES)

    # layouts are pre-swizzled on the host so every DMA row is contiguous
    xt_d = nc.dram_tensor("xt", (2, 128, KT, 512), BF16, kind="ExternalInput")
    ctxt_d = nc.dram_tensor("ctxt", (8, 128, KT, 512), BF16,
                            kind="ExternalInput")
    wqt_d = nc.dram_tensor("wqt", (128, KT, CW), BF16, kind="ExternalInput")
    wkt_d = nc.dram_tensor("wkt", (128, KT, CW), BF16, kind="ExternalInput")
    wvt_d = nc.dram_tensor("wvt", (128, KT, CW), BF16, kind="ExternalInput")
    bq_d = nc.dram_tensor("bq", (128, 2), F32, kind="ExternalInput")
    bk_d = nc.dram_tensor("bk", (128, 2), F32, kind="ExternalInput")
    bv_d = nc.dram_tensor("bv", (128, CW), BF16, kind="ExternalInput")
    out_d = nc.dram_tensor("out", (X, CW), F32, kind="ExternalOutput")

    with tile.TileContext(nc) as tc, ExitStack() as ctx:
        consts = ctx.enter_context(tc.tile_pool(name="consts", bufs=1))
        pt_pool = ctx.enter_context(tc.tile_pool(name="pt", bufs=4))
        osb_pool = ctx.enter_context(tc.tile_pool(name="osb", bufs=2))
        rc_pool = ctx.enter_context(tc.tile_pool(name="rc", bufs=2))
        # one psum pool for everything except the score tiles:
        # 4 slots x 1 bank (projections, O' accumulators, transposes)
        mp = ctx.enter_context(tc.tile_pool(name="mp", bufs=4, space="PSUM"))
        # score tiles: 2 slots x 2 banks (double-buffered so ScalarE's exp
        # never gates the next score matmul pair)
        st_pool = ctx.enter_context(
            tc.tile_pool(name="st", bufs=2, space="PSUM"))

        # ---- resident SBUF tensors ----
        wq_sb = consts.tile([128, KT, CW], BF16)
        wk_sb = consts.tile([128, KT, CW], BF16)
        wv_sb = consts.tile([128, KT, CW], BF16)
        xt_sb = consts.tile([128, KT, X], BF16)
        ctx_sb = consts.tile([128, KT, T], BF16)
        qt_sb = consts.tile([128, 2, X], BF16)
        kt_sb = consts.tile([128, 2, T], BF16)
        vp_sb = consts.tile([128, TT, CH * (HD + 1)], BF16)  # [.., 260]
        out_sb = consts.tile([128, XTILES, CW], F32)
        bq_sb = consts.tile([128, 2], F32)
        bk_sb = consts.tile([128, 2], F32)
        bv_sb = consts.tile([128, CW], BF16)
        ident = consts.tile([128, 128], F32)

        vp_h = vp_sb[:].rearrange("p t (h c) -> p t h c", c=HD + 1)
        bv_h = bv_sb[:].rearrange("p (h c) -> p h c", c=HD)

        # ---- PE warm-up: dummy matmuls while input DMAs land (HAM) ----
        dumin = consts.tile([128, 512], BF16)
        nc.gpsimd.memset(dumin[:], 0.0)
        dps = mp.tile([128, 512], F32, tag="mp", name="dps")
        for i in range(10):
            nc.tensor.matmul(dps[:], dumin[:, 0:128], dumin[:],
                             start=(i == 0), stop=(i == 9))

        # ---- input DMAs (ordered so compute can start early) ----
        def ctx_dma(c):
            nc.sync.dma_start(ctx_sb[:, :, 512 * c:512 * (c + 1)],
                              ctxt_d.ap()[c])

        nc.sync.dma_start(xt_sb[:, :, 0:512], xt_d.ap()[0])
        nc.sync.dma_start(wq_sb[:], wqt_d.ap())
        nc.sync.dma_start(wk_sb[:], wkt_d.ap())
        ctx_dma(0)
        nc.sync.dma_start(wv_sb[:], wvt_d.ap())
        nc.sync.dma_start(bq_sb[:], bq_d.ap())
        nc.sync.dma_start(bk_sb[:], bk_d.ap())
        nc.sync.dma_start(bv_sb[:], bv_d.ap())
        ctx_dma(1)
        nc.sync.dma_start(xt_sb[:, :, 512:1024], xt_d.ap()[1])
        for c in range(2, 8):
            ctx_dma(c)
        make_identity(nc, ident[:])
        nc.gpsimd.memset(vp_h[:, :, :, HD:HD + 1], 1.0)

        # ---- qT projection: [col, x] per (col-tile, x-chunk) ----
        def qt_proj(ct, xc):
            ps = mp.tile([128, 512], F32, tag="mp", name=f"qps{ct}{xc}")
            for dt in range(KT):
                nc.tensor.matmul(
                    ps[:],
                    wq_sb[:, dt, 128 * ct:128 * (ct + 1)],
                    xt_sb[:, dt, 512 * xc:512 * (xc + 1)],
                    start=(dt == 0), stop=(dt == KT - 1))
            nc.vector.tensor_scalar_add(
                qt_sb[:, ct, 512 * xc:512 * (xc + 1)], ps[:],
                bq_sb[:, ct:ct + 1])

        qt_proj(0, 0)   # the other three slices ride later as stream filler

        def kt_chunk(ct, c):
            ps = mp.tile([128, 512], F32, tag="mp", name=f"kps{ct}_{c}")
            for dt in range(KT):
                nc.tensor.matmul(
                    ps[:],
                    wk_sb[:, dt, 128 * ct:128 * (ct + 1)],
                    ctx_sb[:, dt, 512 * c:512 * (c + 1)],
                    start=(dt == 0), stop=(dt == KT - 1))
            nc.vector.tensor_scalar_add(
                kt_sb[:, ct, 512 * c:512 * (c + 1)], ps[:],
                bk_sb[:, ct:ct + 1])

        def v_tile(tt):
            ps = mp.tile([128, 512], F32, tag="mp", name=f"vps{tt}")
            for dt in range(KT):
                nc.tensor.matmul(
                    ps[:, 0:CW],
                    ctx_sb[:, dt, 128 * tt:128 * (tt + 1)],
                    wv_sb[:, dt, :],
                    start=(dt == 0), stop=(dt == KT - 1))
            nc.vector.tensor_add(
                vp_h[:, tt, :, 0:HD],
                ps[:, 0:CW].rearrange("p (h c) -> p h c", c=HD),
                bv_h[:])

        # attention state
        oacc = {}     # (hp, xc) -> [tileA, tileB]

        def attn_start(hp, xc):
            oacc[(hp, xc)] = [
                mp.tile([65, 512], F32, tag="mp", name=f"oacc{hp}{xc}{h2}")
                for h2 in range(2)]

        def attn_step(hp, xc, tt):
            st = st_pool.tile([128, 1024], F32, tag="st", name=f"st{hp}{xc}{tt}")
            for h2 in range(2):
                nc.tensor.matmul(
                    st[:, 512 * h2:512 * (h2 + 1)],
                    kt_sb[64 * h2:64 * (h2 + 1), hp,
                          128 * tt:128 * (tt + 1)],
                    qt_sb[64 * h2:64 * (h2 + 1), hp,
                          512 * xc:512 * (xc + 1)],
                    start=True, stop=True)
            pt = pt_pool.tile([128, 1024], BF16, tag="pt", name=f"pt{hp}{xc}{tt}")
            nc.scalar.activation(pt[:], st[:], EXP, scale=0.125)
            for h2 in range(2):
                h = 2 * hp + h2
                nc.tensor.matmul(
                    oacc[(hp, xc)][h2][:],
                    vp_sb[:, tt, 65 * h:65 * (h + 1)],
                    pt[:, 512 * h2:512 * (h2 + 1)],
                    start=(tt == 0), stop=(tt == TT - 1))

        def attn_drain(hp, xc, out_ap=None):
            ots = []
            for h2 in range(2):
                ot = osb_pool.tile([65, 512], F32, tag="osb", name=f"ot{hp}{xc}{h2}")
                nc.vector.tensor_copy(ot[:], oacc[(hp, xc)][h2][:])
                ots.append(ot)
            for s in range(4):
                for h2 in range(2):
                    h = 2 * hp + h2
                    tp = mp.tile([128, 65], F32, tag="mp", name=f"tp{hp}{xc}{h2}{s}")
                    nc.tensor.transpose(
                        tp[:], ots[h2][:, 128 * s:128 * (s + 1)],
                        ident[0:65, 0:65])
                    rc = rc_pool.tile([128, 1], F32, tag="rc", name=f"rc{hp}{xc}{h2}{s}")
                    nc.vector.reciprocal(rc[:], tp[:, 64:65])
                    nc.vector.tensor_scalar_mul(
                        out_sb[:, 4 * xc + s, 64 * h:64 * (h + 1)],
                        tp[:, 0:64], rc[:])
                if out_ap is not None:
                    # this stream completes x-tile 4*xc+s: ship it out now
                    nc.sync.dma_start(out_ap[:, 4 * xc + s:4 * xc + s + 1],
                                      out_sb[:, 4 * xc + s:4 * xc + s + 1])
            del oacc[(hp, xc)]

        # One attention stream (hp, xc) at a time; PE filler work
        # (kT chunks, v tiles, qT ct1) rides inside the streams so
        # ScalarE's exp stays busy end to end. Each stream's drain is
        # deferred into the next stream's first steps to hide the
        # inter-stream bubble (the freed O' accumulators supply the
        # PSUM slots the drain's transposes need).
        out_ap = out_d.ap().rearrange("(xt p) c -> p xt c", p=128)

        # stream (0,0): kT ct0 chunk-paced + v paced + qT(0,1) +
        # kT ct1 chunk 0
        attn_start(0, 0)
        for c in range(8):
            kt_chunk(0, c)
            for tt in range(4 * c, 4 * c + 4):
                v_tile(tt)
                attn_step(0, 0, tt)
                if tt == 18:
                    qt_proj(0, 1)
            if c == 7:
                kt_chunk(1, 0)

        # stream (0,1): drain of (0,0) overlapped, kT ct1 chunks 1-3,
        # qT(1,0)
        attn_start(0, 1)
        for tt in range(TT):
            attn_step(0, 1, tt)
            if tt == 2:
                attn_drain(0, 0)
            elif tt in (4, 12, 20):
                kt_chunk(1, 1 + (tt - 4) // 8)
            elif tt == 26:
                qt_proj(1, 0)

        # stream (1,0): kT ct1 chunks 4-7 paced (needed from step 16 on),
        # qT(1,1)
        attn_start(1, 0)
        for tt in range(TT):
            attn_step(1, 0, tt)
            if tt == 2:
                attn_drain(0, 1)
            elif tt in (4, 8, 12, 15):
                kt_chunk(1, 4 + [4, 8, 12, 15].index(tt))
            elif tt == 20:
                qt_proj(1, 1)

        # stream (1,1)
        attn_start(1, 1)
        for tt in range(TT):
            attn_step(1, 1, tt)
            if tt == 2:
                attn_drain(1, 0, out_ap)
        attn_drain(1, 1, out_ap)

    nc.compile()
    return nc


def get_program():
    if "nc" not in _CACHE:
        _CACHE["nc"] = _build_program()
    return _CACHE["nc"]


def _swizzle(at, inner):
    """(D, M) d-major -> (M//inner, 128, KT, inner): chunked, partition-
    contiguous rows so each DMA descriptor is a long linear run."""
    dd, m = at.shape
    return np.ascontiguousarray(
        at.reshape(KT, 128, m // inner, inner).transpose(2, 1, 0, 3))


def _shard_inputs(previous_output, context, Wq, bq, Wk, bk, Wv, bv):
    bf = ml_dtypes.bfloat16
    xt = [_swizzle(previous_output[n].T.astype(bf), 512) for n in range(N)]
    ctxt = [_swizzle(context[n].T.astype(bf), 512) for n in range(N)]
    in_maps = []
    for c in range(NCORES):
        n, hg = c // CH, c % CH
        sl = slice(CW * hg, CW * (hg + 1))
        in_maps.append({
            "xt": xt[n],
            "ctxt": ctxt[n],
            "wqt": _swizzle(Wq[sl].T.astype(bf), CW)[0],
            "wkt": _swizzle(Wk[sl].T.astype(bf), CW)[0],
            "wvt": _swizzle(Wv[sl].T.astype(bf), CW)[0],
            "bq": np.ascontiguousarray(
                bq[sl].reshape(2, 128).T).astype(np.float32),
            "bk": np.ascontiguousarray(
                bk[sl].reshape(2, 128).T).astype(np.float32),
            "bv": np.broadcast_to(
                bv[sl].astype(bf), (128, CW)).copy(),
        })
    return in_maps


LAST_RESULTS = None


def kernel(previous_output, context, Wq, bq, Wk, bk, Wv, bv):
    global LAST_RESULTS
    previous_output = np.asarray(previous_output, dtype=np.float32)
    context = np.asarray(context, dtype=np.float32)
    Wq = np.asarray(Wq, dtype=np.float32)
    Wk = np.asarray(Wk, dtype=np.float32)
    Wv = np.asarray(Wv, dtype=np.float32)
    bq = np.asarray(bq, dtype=np.float32)
    bk = np.asarray(bk, dtype=np.float32)
    bv = np.asarray(bv, dtype=np.float32)

    nc = get_program()
    in_maps = _shard_inputs(previous_output, context, Wq, bq, Wk, bk, Wv, bv)
    res = run_bass_kernel_spmd(nc, in_maps, core_ids=list(range(NCORES)))
    LAST_RESULTS = res

    out = np.empty((N, X, D), dtype=np.float32)
    for c in range(NCORES):
        n, hg = c // CH, c % CH
        out[n, :, CW * hg:CW * (hg + 1)] = res.results[c]["out"]
    return out


# revision 22
# speedup vs baseline: 1.0551x; 1.0251x over previous
"""Cross-attention kernel for 8 Trainium2 NeuronCores.

Problem: nn_CrossAttention (N=2, X=1024, T=4096, D=1024, H=16, hd=64).

Sharding: core c handles batch n = c//4 and head-group hg = c%4
(4 heads = 256 output dims). No cross-core communication.

Host prep per core (numpy, outside HW timing):
  - xT   = previous_output[n].T          (D, X)  bf16
  - ctxT = context[n].T                  (D, T)  bf16
  - w{q,k,v}T = W[256*hg:256*(hg+1)].T   (D, 256) bf16
  - biases sliced per core (bv replicated to 128 partitions).

Device (all matmuls contract over the partition dim):
  qT[c,x]  = wqT.T @ xT    (+bq)         kT[c,t] = wkT.T @ ctxT (+bk)
  v[t,c]   = ctxT.T @ wvT  (+bv via DVE broadcast add)
  S.T[t,x] = kT_h.T @ qT_h   (per head, K=64, head pairs packed into
                              array row-halves via base_partition)
  P.T      = exp(S.T / 8)                 (ScalarE, scale folded in)
  O'.T[65,x] = [V_h | 1].T @ P.T          (ones col gives softmax denom)
  O[x,64]  = transpose(O'.T) rows 0:64 * 1/row64   (PE transpose + DVE)

The program is emitted as one software pipeline so exp (ScalarE,
~147us/core total) overlaps the PE work (~165us/core total):
  era1: per ctx chunk c: kT ct0 chunk + per tt: v[tt] + attention
        steps (hp=0, xc=0) and (hp=0, xc=1)
  era2: per tt: kT ct1 pacing + attention steps (hp=1, xc=0/1)
"""

import numpy as np
import ml_dtypes
from contextlib import ExitStack

import concourse.bass as bass
import concourse.bacc as bacc
import concourse.tile as tile
import concourse.mybir as mybir
from concourse.bass_utils import run_bass_kernel_spmd
from concourse.masks import make_identity

D, H, HD = 1024, 16, 64
N, X, T = 2, 1024, 4096
NCORES = 8
CH = 4            # heads per core
CW = CH * HD      # 256 output cols per core
KT = D // 128     # 8 d-tiles
TT = T // 128     # 32 t-tiles
XTILES = X // 128  # 8 x-tiles
BF16 = mybir.dt.bfloat16
F32 = mybir.dt.float32
EXP = mybir.ActivationFunctionType.Exp

_CACHE = {}


def _build_program():
    nc = bacc.Bacc("TRN2", target_bir_lowering=False, debug=False,
                   num_devices=NCORES)

    # layouts are pre-swizzled on the host so every DMA row is contiguous
    xt_d = nc.dram_tensor("xt", (2, 128, KT, 512), BF16, kind="ExternalInput")
    ctxt_d = nc.dram_tensor("ctxt", (8, 128, KT, 512), BF16,
                            kind="ExternalInput")
    wqt_d = nc.dram_tensor("wqt", (128, KT, CW), BF16, kind="ExternalInput")
    wkt_d = nc.dram_tensor("wkt", (128, KT, CW), BF16, kind="ExternalInput")
    wvt_d = nc.dram_tensor("wvt", (128, KT, CW), BF16, kind="ExternalInput")
    bq_d = nc.dram_tensor("bq", (128, 2), F32, kind="ExternalInput")
    bk_d = nc.dram_tensor("bk", (128, 2), F32, kind="ExternalInput")
    bv_d = nc.dram_tensor("bv", (128, CW), BF16, kind="ExternalInput")
    out_d = nc.dram_tensor("out", (X, CW), F32, kind="ExternalOutput")

    with tile.TileContext(nc) as tc, ExitStack() as ctx:
        consts = ctx.enter_context(tc.tile_pool(name="consts", bufs=1))
        pt_pool = ctx.enter_context(tc.tile_pool(name="pt", bufs=4))
        osb_pool = ctx.enter_context(tc.tile_pool(name="osb", bufs=2))
        rc_pool = ctx.enter_context(tc.tile_pool(name="rc", bufs=2))
        # one psum pool for everything except the score tiles:
        # 4 slots x 1 bank (projections, O' accumulators, transposes)
        mp = ctx.enter_context(tc.tile_pool(name="mp", bufs=4, space="PSUM"))
        # score tiles: 2 slots x 2 banks (double-buffered so ScalarE's exp
        # never gates the next score matmul pair)
        st_pool = ctx.enter_context(
            tc.tile_pool(name="st", bufs=2, space="PSUM"))

        # ---- resident SBUF tensors ----
        wq_sb = consts.tile([128, KT, CW], BF16)
        wk_sb = consts.tile([128, KT, CW], BF16)
        wv_sb = consts.tile([128, KT, CW], BF16)
        xt_sb = consts.tile([128, KT, X], BF16)
        ctx_sb = consts.tile([128, KT, T], BF16)
        qt_sb = consts.tile([128, 2, X], BF16)
        kt_sb = consts.tile([128, 2, T], BF16)
        vp_sb = consts.tile([128, TT, CH * (HD + 1)], BF16)  # [.., 260]
        out_sb = consts.tile([128, XTILES, CW], F32)
        bq_sb = consts.tile([128, 2], F32)
        bk_sb = consts.tile([128, 2], F32)
        bv_sb = consts.tile([128, CW], BF16)
        ident = consts.tile([128, 128], F32)

        vp_h = vp_sb[:].rearrange("p t (h c) -> p t h c", c=HD + 1)
        bv_h = bv_sb[:].rearrange("p (h c) -> p h c", c=HD)

        # ---- PE warm-up: dummy matmuls while input DMAs land (HAM) ----
        dumin = consts.tile([128, 512], BF16)
        nc.gpsimd.memset(dumin[:], 0.0)
        dps = mp.tile([128, 512], F32, tag="mp", name="dps")
        for i in range(10):
            nc.tensor.matmul(dps[:], dumin[:, 0:128], dumin[:],
                             start=(i == 0), stop=(i == 9))

        # ---- input DMAs (ordered so compute can start early) ----
        def ctx_dma(c):
            nc.sync.dma_start(ctx_sb[:, :, 512 * c:512 * (c + 1)],
                              ctxt_d.ap()[c])

        nc.sync.dma_start(xt_sb[:, :, 0:512], xt_d.ap()[0])
        nc.sync.dma_start(wq_sb[:], wqt_d.ap())
        nc.sync.dma_start(wk_sb[:], wkt_d.ap())
        ctx_dma(0)
        nc.sync.dma_start(wv_sb[:], wvt_d.ap())
        nc.sync.dma_start(bq_sb[:], bq_d.ap())
        nc.sync.dma_start(bk_sb[:], bk_d.ap())
        nc.sync.dma_start(bv_sb[:], bv_d.ap())
        ctx_dma(1)
        nc.sync.dma_start(xt_sb[:, :, 512:1024], xt_d.ap()[1])
        for c in range(2, 8):
            ctx_dma(c)
        make_identity(nc, ident[:])
        nc.gpsimd.memset(vp_h[:, :, :, HD:HD + 1], 1.0)

        # ---- qT projection: [col, x] per (col-tile, x-chunk) ----
        def qt_proj(ct, xc):
            ps = mp.tile([128, 512], F32, tag="mp", name=f"qps{ct}{xc}")
            for dt in range(KT):
                nc.tensor.matmul(
                    ps[:],
                    wq_sb[:, dt, 128 * ct:128 * (ct + 1)],
                    xt_sb[:, dt, 512 * xc:512 * (xc + 1)],
                    start=(dt == 0), stop=(dt == KT - 1))
            nc.vector.tensor_scalar_add(
                qt_sb[:, ct, 512 * xc:512 * (xc + 1)], ps[:],
                bq_sb[:, ct:ct + 1])

        qt_proj(0, 0)   # the other three slices ride later as stream filler

        def kt_chunk(ct, c):
            ps = mp.tile([128, 512], F32, tag="mp", name=f"kps{ct}_{c}")
            for dt in range(KT):
                nc.tensor.matmul(
                    ps[:],
                    wk_sb[:, dt, 128 * ct:128 * (ct + 1)],
                    ctx_sb[:, dt, 512 * c:512 * (c + 1)],
                    start=(dt == 0), stop=(dt == KT - 1))
            nc.vector.tensor_scalar_add(
                kt_sb[:, ct, 512 * c:512 * (c + 1)], ps[:],
                bk_sb[:, ct:ct + 1])

        def v_tile(tt):
            ps = mp.tile([128, 512], F32, tag="mp", name=f"vps{tt}")
            for dt in range(KT):
                nc.tensor.matmul(
                    ps[:, 0:CW],
                    ctx_sb[:, dt, 128 * tt:128 * (tt + 1)],
                    wv_sb[:, dt, :],
                    start=(dt == 0), stop=(dt == KT - 1))
            nc.vector.tensor_add(
                vp_h[:, tt, :, 0:HD],
                ps[:, 0:CW].rearrange("p (h c) -> p h c", c=HD),
                bv_h[:])

        # attention state
        oacc = {}     # (hp, xc) -> [tileA, tileB]

        def attn_start(hp, xc):
            oacc[(hp, xc)] = [
                mp.tile([65, 512], F32, tag="mp", name=f"oacc{hp}{xc}{h2}")
                for h2 in range(2)]

        def attn_step(hp, xc, tt):
            st = st_pool.tile([128, 1024], F32, tag="st", name=f"st{hp}{xc}{tt}")
            for h2 in range(2):
                nc.tensor.matmul(
                    st[:, 512 * h2:512 * (h2 + 1)],
                    kt_sb[64 * h2:64 * (h2 + 1), hp,
                          128 * tt:128 * (tt + 1)],
                    qt_sb[64 * h2:64 * (h2 + 1), hp,
                          512 * xc:512 * (xc + 1)],
                    start=True, stop=True)
            pt = pt_pool.tile([128, 1024], BF16, tag="pt", name=f"pt{hp}{xc}{tt}")
            nc.scalar.activation(pt[:], st[:], EXP, scale=0.125)
            for h2 in range(2):
                h = 2 * hp + h2
                nc.tensor.matmul(
                    oacc[(hp, xc)][h2][:],
                    vp_sb[:, tt, 65 * h:65 * (h + 1)],
                    pt[:, 512 * h2:512 * (h2 + 1)],
                    start=(tt == 0), stop=(tt == TT - 1))

        def attn_drain(hp, xc, out_ap=None):
            ots = []
            for h2 in range(2):
                ot = osb_pool.tile([65, 512], F32, tag="osb", name=f"ot{hp}{xc}{h2}")
                nc.vector.tensor_copy(ot[:], oacc[(hp, xc)][h2][:])
                ots.append(ot)
            for s in range(4):
                for h2 in range(2):
                    h = 2 * hp + h2
                    tp = mp.tile([128, 65], F32, tag="mp", name=f"tp{hp}{xc}{h2}{s}")
                    nc.tensor.transpose(
                        tp[:], ots[h2][:, 128 * s:128 * (s + 1)],
                        ident[0:65, 0:65])
                    rc = rc_pool.tile([128, 1], F32, tag="rc", name=f"rc{hp}{xc}{h2}{s}")
                    nc.vector.reciprocal(rc[:], tp[:, 64:65])
                    nc.vector.tensor_scalar_mul(
                        out_sb[:, 4 * xc + s, 64 * h:64 * (h + 1)],
                        tp[:, 0:64], rc[:])
                if out_ap is not None:
                    # this stream completes x-tile 4*xc+s: ship it out now
                    nc.sync.dma_start(out_ap[:, 4 * xc + s:4 * xc + s + 1],
                                      out_sb[:, 4 * xc + s:4 * xc + s + 1])
            del oacc[(hp, xc)]

        # One attention stream (hp, xc) at a time; PE filler work
        # (kT chunks, v tiles, qT ct1) rides inside the streams so
        # ScalarE's exp stays busy end to end. Each stream's drain is
        # deferred into the next stream's first steps to hide the
        # inter-stream bubble (the freed O' accumulators supply the
        # PSUM slots the drain's transposes need).
        out_ap = out_d.ap().rearrange("(xt p) c -> p xt c", p=128)

        # stream (0,0): kT ct0 chunk-paced + v paced + qT(0,1) +
        # kT ct1 chunk 0
        attn_start(0, 0)
        for c in range(8):
            kt_chunk(0, c)
            for tt in range(4 * c, 4 * c + 4):
                v_tile(tt)
                attn_step(0, 0, tt)
                if tt == 18:
                    qt_proj(0, 1)
            if c == 7:
                kt_chunk(1, 0)

        # stream (0,1): drain of (0,0) overlapped, kT ct1 chunks 1-3,
        # qT(1,0)
        attn_start(0, 1)
        for tt in range(TT):
            attn_step(0, 1, tt)
            if tt == 6:
                attn_drain(0, 0)
            elif tt in (8, 16, 24):
                kt_chunk(1, 1 + (tt - 8) // 8)
            elif tt == 28:
                qt_proj(1, 0)

        # stream (1,0): kT ct1 chunks 4-7 paced (needed from step 16 on),
        # qT(1,1)
        attn_start(1, 0)
        for tt in range(TT):
            attn_step(1, 0, tt)
            if tt == 6:
                attn_drain(0, 1)
            elif tt in (8, 11, 14, 15):
                kt_chunk(1, 4 + [8, 11, 14, 15].index(tt))
            elif tt == 20:
                qt_proj(1, 1)

        # stream (1,1)
        attn_start(1, 1)
        for tt in range(TT):
            attn_step(1, 1, tt)
            if tt == 6:
                attn_drain(1, 0, out_ap)
        attn_drain(1, 1, out_ap)

    nc.compile()
    return nc


def get_program():
    if "nc" not in _CACHE:
        _CACHE["nc"] = _build_program()
    return _CACHE["nc"]


def _swizzle(at, inner):
    """(D, M) d-major -> (M//inner, 128, KT, inner): chunked, partition-
    contiguous rows so each DMA descriptor is a long linear run."""
    dd, m = at.shape
    return np.ascontiguousarray(
        at.reshape(KT, 128, m // inner, inner).transpose(2, 1, 0, 3))


def _shard_inputs(previous_output, context, Wq, bq, Wk, bk, Wv, bv):
    bf = ml_dtypes.bfloat16
    xt = [_swizzle(previous_output[n].T.astype(bf), 512) for n in range(N)]
    ctxt = [_swizzle(context[n].T.astype(bf), 512) for n in range(N)]
    in_maps = []
    for c in range(NCORES):
        n, hg = c // CH, c % CH
        sl = slice(CW * hg, CW * (hg + 1))
        in_maps.append({
            "xt": xt[n],
            "ctxt": ctxt[n],
            "wqt": _swizzle(Wq[sl].T.astype(bf), CW)[0],
            "wkt": _swizzle(Wk[sl].T.astype(bf), CW)[0],
            "wvt": _swizzle(Wv[sl].T.astype(bf), CW)[0],
            "bq": np.ascontiguousarray(
                bq[sl].reshape(2, 128).T).astype(np.float32),
            "bk": np.ascontiguousarray(
                bk[sl].reshape(2, 128).T).astype(np.float32),
            "bv": np.broadcast_to(
                bv[sl].astype(bf), (128, CW)).copy(),
        })
    return in_maps


LAST_RESULTS = None


def kernel(previous_output, context, Wq, bq, Wk, bk, Wv, bv):
    global LAST_RESULTS
    previous_output = np.asarray(previous_output, dtype=np.float32)
    context = np.asarray(context, dtype=np.float32)
    Wq = np.asarray(Wq, dtype=np.float32)
    Wk = np.asarray(Wk, dtype=np.float32)
    Wv = np.asarray(Wv, dtype=np.float32)
    bq = np.asarray(bq, dtype=np.float32)
    bk = np.asarray(bk, dtype=np.float32)
    bv = np.asarray(bv, dtype=np.float32)

    nc = get_program()
    in_maps = _shard_inputs(previous_output, context, Wq, bq, Wk, bk, Wv, bv)
    res = run_bass_kernel_spmd(nc, in_maps, core_ids=list(range(NCORES)))
    LAST_RESULTS = res

    out = np.empty((N, X, D), dtype=np.float32)
    for c in range(NCORES):
        n, hg = c // CH, c % CH
        out[n, :, CW * hg:CW * (hg + 1)] = res.results[c]["out"]
    return out
